# revision 8
# baseline (speedup 1.0000x reference)
"""Trainium2 Bass kernel for the GNN message-passing net (nn_Net_76690936037575).

Math: with assoc_var == arange(n_var) and assoc_con == arange(n_con) (the
spec-guaranteed fill), the scatter/scatter/gather pipeline collapses to

    out[0:n_con]      = head(con_mlp(con_node_features))
    out[n_con:n_var]  = head(var_mlp(var_node_features[n_con:n_var]))

where head/a 2-layer mlp have no nonlinearity between mlp's 2nd matmul and
head's 1st, so those two 128x128 matmuls fuse on the host:
    M_s = sW2 @ W1,  bM_s = sb2 @ W1 + b1        (s in {var, con})

Per 512-row tile the device does 5 matmuls (K=2 in-layer; 3x 128x128; 128->1
out-layer) in float32r, with bias+ReLU fused into ScalarE/VectorE ops. The
128->1 matmuls of 4 consecutive tiles are col-tiled into one PSUM bank at
partitions {0,32,64,96} so a single Sigmoid instruction covers 4 tiles.

Sharding: node rows split evenly across 8 cores (con: 50k/core, var:
25k/core), weights replicated.
"""

import math

import ml_dtypes
import numpy as np

DIM = 128
TILE_N = 512  # rows per matmul tile (one PSUM bank of f32)
GROUP = 4  # tiles per sigmoid group (col-tiled final matmuls)
N_CORES = 8

_NC_CACHE = {}


def _build_nc(ncon, nvar):
    """Build the per-core Bass program. ncon/nvar: rows per core."""
    import concourse.mybir as mybir
    import concourse.tile as tile
    from concourse import bacc

    dt = mybir.dt
    f32 = dt.float32
    f32r = dt.float32r
    bf16 = dt.bfloat16
    AF = mybir.ActivationFunctionType
    ALU = mybir.AluOpType

    nc = bacc.Bacc()

    cfeat = nc.dram_tensor("cfeat", [2, ncon], f32r, kind="ExternalInput")
    vfeat = nc.dram_tensor("vfeat", [2, nvar], f32r, kind="ExternalInput")
    wa1c = nc.dram_tensor("wa1c", [2, DIM], f32r, kind="ExternalInput")
    wa1v = nc.dram_tensor("wa1v", [2, DIM], f32r, kind="ExternalInput")
    wmc = nc.dram_tensor("wmc", [DIM, DIM], f32r, kind="ExternalInput")
    wmv = nc.dram_tensor("wmv", [DIM, DIM], f32r, kind="ExternalInput")
    ww2 = nc.dram_tensor("ww2", [DIM, DIM], f32r, kind="ExternalInput")
    ww3 = nc.dram_tensor("ww3", [DIM, DIM], f32r, kind="ExternalInput")
    ww4 = nc.dram_tensor("ww4", [DIM, 32], bf16, kind="ExternalInput")
    bb1c = nc.dram_tensor("bb1c", [DIM, 1], f32, kind="ExternalInput")
    bb1v = nc.dram_tensor("bb1v", [DIM, 1], f32, kind="ExternalInput")
    bbmc = nc.dram_tensor("bbmc", [DIM, 1], f32, kind="ExternalInput")
    bbmv = nc.dram_tensor("bbmv", [DIM, 1], f32, kind="ExternalInput")
    bb2 = nc.dram_tensor("bb2", [DIM, 1], f32, kind="ExternalInput")
    bb3 = nc.dram_tensor("bb3", [DIM, 1], f32, kind="ExternalInput")
    bb4 = nc.dram_tensor("bb4", [DIM, 1], f32, kind="ExternalInput")
    out_con = nc.dram_tensor("out_con", [ncon], f32, kind="ExternalOutput")
    out_var = nc.dram_tensor("out_var", [nvar], f32, kind="ExternalOutput")

    with tile.TileContext(nc) as tc:
        with (
            tc.tile_pool(name="const", bufs=1) as cpool,
            tc.tile_pool(name="feat", bufs=3) as fpool,
            tc.tile_pool(name="acts", bufs=12) as apool,
            tc.tile_pool(name="sig", bufs=2) as spool,
            tc.tile_pool(name="mm", bufs=6, space="PSUM") as mmpool,
            tc.tile_pool(name="p5", bufs=2, space="PSUM") as p5pool,
        ):

            def cload(dram, shape, tag, cdt=f32):
                t = cpool.tile(shape, cdt, tag=tag)
                nc.sync.dma_start(t[:, :], dram[:, :])
                return t

            wa1c_t = cload(wa1c, [2, DIM], "wa1c", f32r)
            wa1v_t = cload(wa1v, [2, DIM], "wa1v", f32r)
            wmc_t = cload(wmc, [DIM, DIM], "wmc", f32r)
            wmv_t = cload(wmv, [DIM, DIM], "wmv", f32r)
            ww2_t = cload(ww2, [DIM, DIM], "ww2", f32r)
            ww3_t = cload(ww3, [DIM, DIM], "ww3", f32r)
            ww4_t = cload(ww4, [DIM, 32], "ww4", bf16)
            bb1c_t = cload(bb1c, [DIM, 1], "bb1c")
            bb1v_t = cload(bb1v, [DIM, 1], "bb1v")
            bbmc_t = cload(bbmc, [DIM, 1], "bbmc")
            bbmv_t = cload(bbmv, [DIM, 1], "bbmv")
            bb2_t = cload(bb2, [DIM, 1], "bb2")
            bb3_t = cload(bb3, [DIM, 1], "bb3")
            bb4_t = cload(bb4, [DIM, 1], "bb4")

            def do_stream(feat, n_rows, wa1_t, b1_t, wm_t, bm_t, out):
                n_tiles = math.ceil(n_rows / TILE_N)
                n_groups = math.ceil(n_tiles / GROUP)
                for g in range(n_groups):
                    tiles = list(range(g * GROUP, min((g + 1) * GROUP, n_tiles)))
                    g0 = g * GROUP * TILE_N
                    g_rows = min(GROUP * TILE_N, n_rows - g0)
                    njs = [min(TILE_N, n_rows - t * TILE_N) for t in tiles]
                    full = all(n == TILE_N for n in njs)

                    ftile = fpool.tile([2, GROUP * TILE_N], f32r, tag="feat")
                    nc.sync.dma_start(ftile[:, :g_rows], feat[:, g0 : g0 + g_rows])

                    # layer 1: h = relu(x @ A1 + a1), K=2
                    p1s = []
                    for j, _ in enumerate(tiles):
                        nj = njs[j]
                        p1 = mmpool.tile([DIM, TILE_N], f32, tag="mm")
                        nc.tensor.matmul(
                            p1[:, :nj],
                            wa1_t[:, :],
                            ftile[:, j * TILE_N : j * TILE_N + nj],
                            start=True,
                            stop=True,
                        )
                        p1s.append(p1)
                    t1s = []
                    for j, _ in enumerate(tiles):
                        nj = njs[j]
                        t1 = apool.tile([DIM, TILE_N], f32r, tag="acts")
                        nc.scalar.activation(
                            t1[:, :nj], p1s[j][:, :nj], AF.Relu, bias=b1_t[:, :]
                        )
                        t1s.append(t1)

                    # layer 2 (fused mlp2+head1): u = relu(h @ M + bM)
                    p2s = []
                    for j, _ in enumerate(tiles):
                        nj = njs[j]
                        p2 = mmpool.tile([DIM, TILE_N], f32, tag="mm")
                        nc.tensor.matmul(
                            p2[:, :nj],
                            wm_t[:, :],
                            t1s[j][:, :nj],
                            start=True,
                            stop=True,
                        )
                        p2s.append(p2)
                    t2s = []
                    for j, _ in enumerate(tiles):
                        nj = njs[j]
                        t2 = apool.tile([DIM, TILE_N], f32r, tag="acts")
                        nc.vector.tensor_scalar(
                            t2[:, :nj], p2s[j][:, :nj], bm_t[:, :], 0.0,
                            ALU.add, ALU.max,
                        )
                        t2s.append(t2)

                    # layer 3: v = relu(u @ W2 + b2)
                    p3s = []
                    for j, _ in enumerate(tiles):
                        nj = njs[j]
                        p3 = mmpool.tile([DIM, TILE_N], f32, tag="mm")
                        nc.tensor.matmul(
                            p3[:, :nj],
                            ww2_t[:, :],
                            t2s[j][:, :nj],
                            start=True,
                            stop=True,
                        )
                        p3s.append(p3)
                    t3s = []
                    for j, _ in enumerate(tiles):
                        nj = njs[j]
                        t3 = apool.tile([DIM, TILE_N], f32r, tag="acts")
                        nc.scalar.activation(
                            t3[:, :nj], p3s[j][:, :nj], AF.Relu, bias=bb2_t[:, :]
                        )
                        t3s.append(t3)

                    # layer 4: w = relu(v @ W3 + b3)
                    p4s = []
                    for j, _ in enumerate(tiles):
                        nj = njs[j]
                        p4 = mmpool.tile([DIM, TILE_N], f32, tag="mm")
                        nc.tensor.matmul(
                            p4[:, :nj],
                            ww3_t[:, :],
                            t3s[j][:, :nj],
                            start=True,
                            stop=True,
                        )
                        p4s.append(p4)
                    t4s = []
                    for j, _ in enumerate(tiles):
                        nj = njs[j]
                        t4 = apool.tile([DIM, TILE_N], bf16, tag="t4")
                        nc.vector.tensor_scalar(
                            t4[:, :nj], p4s[j][:, :nj], bb3_t[:, :], 0.0,
                            ALU.add, ALU.max,
                        )
                        t4s.append(t4)

                    # layer 5: y = sigmoid(w @ W4 + b4), col-tiled 4 tiles/bank
                    p5 = p5pool.tile([DIM, TILE_N], f32, tag="p5")
                    for j, _ in enumerate(tiles):
                        nj = njs[j]
                        nc.tensor.matmul(
                            p5[32 * j : 32 * j + 32, :nj],
                            ww4_t[:, :],
                            t4s[j][:, :nj],
                            start=True,
                            stop=True,
                            tile_position=(0, 32 * j),
                        )
                    sg = spool.tile([DIM, TILE_N], f32, tag="sig")
                    if full:
                        pcov = 32 * len(tiles)
                        pmax = 32 * (len(tiles) - 1) + 1
                        nc.scalar.activation(
                            sg[:pcov, :], p5[:pcov, :], AF.Sigmoid,
                            bias=bb4_t[:pcov, :],
                        )
                        nc.sync.dma_start(
                            out[g0 : g0 + g_rows].rearrange("(a b) -> a b", b=TILE_N),
                            sg[0:pmax:32, :],
                        )
                    else:
                        for j, t in enumerate(tiles):
                            nj = njs[j]
                            nc.scalar.activation(
                                sg[32 * j : 32 * j + 1, :nj],
                                p5[32 * j : 32 * j + 1, :nj],
                                AF.Sigmoid,
                                bias=bb4_t[32 * j : 32 * j + 1, :],
                            )
                            nc.sync.dma_start(
                                out[t * TILE_N : t * TILE_N + nj].rearrange(
                                    "(a b) -> a b", a=1
                                ),
                                sg[32 * j : 32 * j + 1, :nj],
                            )

            do_stream(cfeat, ncon, wa1c_t, bb1c_t, wmc_t, bbmc_t, out_con)
            do_stream(vfeat, nvar, wa1v_t, bb1v_t, wmv_t, bbmv_t, out_var)

    nc.compile()
    return nc


def _make_in_maps(inputs, ncon_per, nvar_per):
    """Host-side sharding: transpose features, split rows, fuse weights."""
    f32 = np.float32
    cf = np.asarray(inputs["con_node_features"], f32)
    vf = np.asarray(inputs["var_node_features"], f32)
    n_con = cf.shape[0]
    n_var = vf.shape[0]

    W1 = np.asarray(inputs["W1"], f32)
    b1 = np.asarray(inputs["b1"], f32)
    mc = np.asarray(inputs["cW2"], f32) @ W1
    bmc = np.asarray(inputs["cb2"], f32) @ W1 + b1
    mv = np.asarray(inputs["vW2"], f32) @ W1
    bmv = np.asarray(inputs["vb2"], f32) @ W1 + b1

    conT = np.zeros((2, ncon_per * N_CORES), f32)
    conT[:, :n_con] = cf.T
    varT = np.zeros((2, nvar_per * N_CORES), f32)
    varT[:, : n_var - n_con] = vf[n_con:].T

    def col(v):
        return np.ascontiguousarray(np.asarray(v, f32).reshape(DIM, 1))

    shared = {
        "wa1c": np.ascontiguousarray(np.asarray(inputs["cW1"], f32)),
        "wa1v": np.ascontiguousarray(np.asarray(inputs["vW1"], f32)),
        "wmc": np.ascontiguousarray(mc),
        "wmv": np.ascontiguousarray(mv),
        "ww2": np.ascontiguousarray(np.asarray(inputs["W2"], f32)),
        "ww3": np.ascontiguousarray(np.asarray(inputs["W3"], f32)),
        "ww4": np.ascontiguousarray(
            np.repeat(np.asarray(inputs["W4"], f32).reshape(DIM, 1), 32, axis=1)
        ).astype(ml_dtypes.bfloat16),
        "bb1c": col(inputs["cb1"]),
        "bb1v": col(inputs["vb1"]),
        "bbmc": col(bmc),
        "bbmv": col(bmv),
        "bb2": col(inputs["b2"]),
        "bb3": col(inputs["b3"]),
        "bb4": np.full((DIM, 1), np.asarray(inputs["b4"], f32).reshape(-1)[0], f32),
    }
    in_maps = []
    for i in range(N_CORES):
        m = dict(shared)
        m["cfeat"] = np.ascontiguousarray(conT[:, i * ncon_per : (i + 1) * ncon_per])
        m["vfeat"] = np.ascontiguousarray(varT[:, i * nvar_per : (i + 1) * nvar_per])
        in_maps.append(m)
    return in_maps


def _reference_numpy(inputs):
    """General fallback (non-arange assoc indices): plain numpy."""
    f32 = np.float32

    def mlp2(x, W1, b1, W2, b2):
        return np.maximum(x @ W1 + b1, 0.0) @ W2 + b2

    vf = np.asarray(inputs["var_node_features"], f32)
    cf = np.asarray(inputs["con_node_features"], f32)
    av = np.asarray(inputs["assoc_var"])
    ac = np.asarray(inputs["assoc_con"])
    n = mlp2(vf, inputs["vW1"], inputs["vb1"], inputs["vW2"], inputs["vb2"])
    e = mlp2(cf, inputs["cW1"], inputs["cb1"], inputs["cW2"], inputs["cb2"])
    x = np.zeros((np.asarray(inputs["node_types"]).shape[0], n.shape[-1]), f32)
    x[av] = n
    x[ac] = e
    x = x[av]
    x = np.maximum(x @ inputs["W1"] + inputs["b1"], 0.0)
    x = np.maximum(x @ inputs["W2"] + inputs["b2"], 0.0)
    x = np.maximum(x @ inputs["W3"] + inputs["b3"], 0.0)
    x = x @ inputs["W4"] + inputs["b4"]
    return (1.0 / (1.0 + np.exp(-x))).astype(f32).squeeze(-1)


def kernel(**inputs):
    from concourse.bass_utils import run_bass_kernel_spmd

    cf = np.asarray(inputs["con_node_features"])
    vf = np.asarray(inputs["var_node_features"])
    av = np.asarray(inputs["assoc_var"])
    ac = np.asarray(inputs["assoc_con"])
    n_con = cf.shape[0]
    n_var = vf.shape[0]

    fast = (
        n_con <= n_var
        and av.shape[0] == n_var
        and ac.shape[0] == n_con
        and np.array_equal(av, np.arange(n_var, dtype=av.dtype))
        and np.array_equal(ac, np.arange(n_con, dtype=ac.dtype))
    )
    if not fast:
        return _reference_numpy(inputs)

    ncon_per = math.ceil(n_con / N_CORES)
    nvar_per = math.ceil((n_var - n_con) / N_CORES)

    key = (ncon_per, nvar_per)
    if key not in _NC_CACHE:
        _NC_CACHE[key] = _build_nc(ncon_per, nvar_per)
    nc = _NC_CACHE[key]

    in_maps = _make_in_maps(inputs, ncon_per, nvar_per)
    res = run_bass_kernel_spmd(nc, in_maps, core_ids=list(range(N_CORES)))

    out = np.empty(n_var, np.float32)
    oc = np.concatenate([r["out_con"] for r in res.results])
    ov = np.concatenate([r["out_var"] for r in res.results])
    out[:n_con] = oc[:n_con]
    out[n_con:] = ov[: n_var - n_con]
    return out
